# revision 1
# baseline (speedup 1.0000x reference)
"""Trainium2 Bass kernel for nn_Attention_32049045963483 (sparse_attention).

Math collapse (verified vs reference at ~3e-6 rel err):
  - qkv 1x1 conv folds into the 11x11/stride-8 down-convs:
      conv(W1 @ f, wq) == conv(f, w_eff),  w_eff[oc,d] = sum_ic wq[oc,ic] W1[ic,d]
  - nearest-neighbor 64x upsample of the [64,64] score map + softmax over the
    upsampled axis == softmax of the low-res map; with row index i -> i//64 = x,
    every output row depends only on x.
  - v enters only through 64-wide block sums:  vbar[c,J] = sum_y v[c,J,y]
      = Wv @ fbar,  fbar[d,J] = sum_y f[d,J,y]   (v never materializes)
  - out[c,x,y] = (sum_J e[J,x] * vbar[c,J]) / (64 * sum_J e[J,x]),
      e[J,I] = exp(scale * dots[I,J])  -- broadcast along y.

Sharding: head-parallel over 8 cores. Core i computes global channels
8i..8i+7 (head i): conv out-channel slices of wq/wk, v-row slice of w_qkv.
Each core reads full f (the down-convs mix all 64 input channels).

Conv structure: factorized two-stage form so the heavy matmuls stream with
free dim >= 256, where float32r runs at full rate (plain fp32 is 1/4):
  stage 1: s[(ky,oc), r, ox] = sum_d w_eff[d,(ky,oc)]@kx . fpad[d, r, 8ox+kx]
           accumulated over kx, in two r-chunks (B: rows 34..66 first --
           matches DMA arrival order -- then A: rows 0..33)
  stage 2: q_low[oc,(oy,ox)]  = sum_ky s[(ky,oc), 8oy+ky, ox]
           via identity-slice selection stationaries; q and k accumulate in
           separate base-0 PSUM tiles so dots needs no partition rebase.
"""

import numpy as np

N_CORES = 8
SCALE = 8.0 ** -0.5  # dim_head ** -0.5

# packed [64, *] weight tensor columns: [wqr | w1q | wkr | w1k | wvt | bq | bk]
C_WQR = 0
C_W1Q = 968
C_WKR = 1032
C_W1K = 2000
C_WVT = 2064
C_BQ = 2072
C_BK = 2073
C_TOT = 2080

_CACHE = {}

LAST_RESULTS = None  # BassKernelResults of the most recent run (for test harness)


def _dep(after, before, sync=False):
    from concourse.tile import add_dep_helper

    a = getattr(after, "ins", after)
    b = getattr(before, "ins", before)
    add_dep_helper(a, b, sync=sync, reason="pin order")


def _build_nc():
    from contextlib import ExitStack

    import concourse.bacc as bacc
    import concourse.mybir as mybir
    import concourse.tile as tile

    f32 = mybir.dt.float32
    f32r = mybir.dt.float32r
    bf16 = mybir.dt.bfloat16
    X = mybir.AxisListType.X
    AF = mybir.ActivationFunctionType

    # Bacc (not raw Bass): its compile() splits >1-wait sync via event
    # semaphores -- hardware allows only one sync wait per instruction.
    nc = bacc.Bacc("TRN2", target_bir_lowering=False)

    f_d = nc.dram_tensor("f", [64, 68 * 68], f32r, kind="ExternalInput")
    wp_d = nc.dram_tensor("wp", [64, 2064], mybir.dt.float16, kind="ExternalInput")
    w2_d = nc.dram_tensor("w2", [64, 16], f32, kind="ExternalInput")
    ws_d = nc.dram_tensor("ws", [88, 88], f32r, kind="ExternalInput")
    out_d = nc.dram_tensor("out", [8, 4096], f32, kind="ExternalOutput")

    with tile.TileContext(nc) as tc:
        with ExitStack() as ctx:
            sb = ctx.enter_context(tc.tile_pool(name="sb", bufs=1))
            ps = ctx.enter_context(tc.tile_pool(name="ps", bufs=1, space="PSUM"))

            fpad = sb.tile([64, 68 * 68], f32r)
            wp_t = sb.tile([64, 2064], mybir.dt.float16)
            w2_t = sb.tile([64, 16], f32)
            ws_t = sb.tile([88, 88], f32r)
            wmq_t = sb.tile([64, 968], f32r)
            wmk_t = sb.tile([64, 968], f32r)
            sq_t = sb.tile([88, 536], f32r)
            sk_t = sb.tile([88, 536], f32r)
            q_t = sb.tile([8, 64], f32)
            k_t = sb.tile([8, 64], f32)
            e_t = sb.tile([64, 64], f32)
            fbar_t = sb.tile([64, 64], f32)
            vaug_t = sb.tile([64, 9], f32)
            rs_t = sb.tile([64, 1], f32)
            olT_t = sb.tile([64, 8], f32)
            T_t = sb.tile([64, 8 * 64], f32)
            scr_t = sb.tile([1, 1], f32)
            scr2_t = sb.tile([1, 1], f32)

            # --- input DMAs interleaved across both HWDGE rings in priority
            # order: conv weights, f rows 0..33 (A-chunks), f rows 34..67, ws
            fp3 = fpad.rearrange("p (r c) -> p r c", c=68)
            # Strict arrival priority at full 2-ring bandwidth: each f piece's
            # trigger waits on the previous priority class's completion on the
            # OTHER ring (same-ring order is implicit), so wp lands first,
            # then f rows 0..33 (A-chunks), then rows 34..67.
            d_wpA = nc.sync.dma_start(out=wp_t[:, 0:C_WKR], in_=wp_d[:, 0:C_WKR])
            d_wpB = nc.scalar.dma_start(out=wp_t[:, C_WKR:2064], in_=wp_d[:, C_WKR:2064])
            d_fA1 = nc.sync.dma_start(out=fpad[:, 0:1156], in_=f_d[:, 0:1156])
            d_fA2 = nc.scalar.dma_start(out=fpad[:, 1156:2312], in_=f_d[:, 1156:2312])
            d_fB1 = nc.sync.dma_start(out=fpad[:, 2312:3468], in_=f_d[:, 2312:3468])
            d_fB2 = nc.scalar.dma_start(out=fpad[:, 3468:4624], in_=f_d[:, 3468:4624])
            d_ws = nc.sync.dma_start(out=ws_t, in_=ws_d[:])
            d_w2 = nc.sync.dma_start(out=w2_t, in_=w2_d[:])

            # preload ACT function tables during the DMA wait (after the ACT
            # ring's DMA triggers). Exp first, Gelu LAST so the gelu set is
            # resident for the real GELUs; the exp reload hides behind dots.
            nc.vector.memset(scr_t, 0.0)
            nc.vector.memset(vaug_t[:, 8:9], 64.0)
            de = nc.scalar.activation(out=scr2_t, in_=scr_t, func=AF.Exp)
            dg = nc.scalar.activation(out=scr2_t, in_=scr_t, func=AF.Gelu)
            _dep(de, d_wpB)
            _dep(de, d_fB2)
            _dep(dg, de)

            wqr4 = wp_t[:, C_WQR:C_W1Q].rearrange(
                "p (kx ky oc) -> p kx ky oc", ky=11, oc=8
            )
            wkr4 = wp_t[:, C_WKR:C_W1K].rearrange(
                "p (kx ky oc) -> p kx ky oc", ky=11, oc=8
            )
            w1q = wp_t[:, C_W1Q:C_WKR]
            w1k = wp_t[:, C_W1K:C_WVT]
            wvt_v = w2_t[:, 0:8]
            bq_v = w2_t[0:8, 8:9]
            bk_v = w2_t[0:8, 9:10]

            # --- compose conv weights: w_eff[d,(kx,ky,oc)], f32r big-free MMs
            psq = ps.tile([64, 11 * 128], f32, tag="A")
            psk = ps.tile([64, 11 * 128], f32, tag="B")
            psq4 = psq.rearrange("p (kx pad) -> p kx pad", pad=128)
            psk4 = psk.rearrange("p (kx pad) -> p kx pad", pad=128)

            def compose(ps4, w1, wr4):
                for x0, x1 in ((0, 4), (4, 8), (8, 11)):
                    nc.tensor.matmul(
                        ps4[:, x0:x1, 0:88], w1, wr4[:, x0:x1],
                        start=True, stop=True,
                    )

            compose(psq4, w1q, wqr4)
            nc.vector.tensor_copy(out=wmq_t, in_=psq4[:, :, 0:88])
            compose(psk4, w1k, wkr4)
            nc.vector.tensor_copy(out=wmk_t, in_=psk4[:, :, 0:88])

            # --- stage 1: per conv, 11 kx accumulate; free = (r-chunk, ox)
            def s1(pst, wm, sl_r):
                out = []
                for kx in range(11):
                    out.append(nc.tensor.matmul(
                        pst, wm[:, kx * 88 : kx * 88 + 88],
                        fp3[:, sl_r, kx : kx + 57 : 8],
                        start=(kx == 0), stop=(kx == 10),
                    ))
                return out[0]

            slAr, slBr = slice(0, 34), slice(34, 67)

            # fbar sub-reduces interleave into DVE gaps (J in groups of 16);
            # parts 0,1 need only f rows 2..33 which arrive first
            def fbar_part(j):
                return nc.vector.reduce_sum(
                    out=fbar_t[:, 16 * j : 16 * (j + 1)],
                    in_=fp3[:, 2 + 16 * j : 18 + 16 * j, 2:66].bitcast(f32),
                    axis=X,
                )

            gateA = nc.tensor.ldweights(weights=fpad[:, 138:139].bitcast(bf16))
            gateA2 = nc.tensor.ldweights(weights=fpad[:, 1160:1161].bitcast(bf16))
            ps_qA = ps.tile([88, 272], f32, tag="C")
            ps_kA = ps.tile([88, 272], f32, tag="D")
            qa = s1(ps_qA, wmq_t, slAr)
            _dep(qa, gateA)
            _dep(qa, gateA2)
            fb0 = fbar_part(0)
            fb1 = fbar_part(1)
            cast_qA = nc.vector.tensor_copy(out=sq_t[:, 0:272], in_=ps_qA)
            ka = s1(ps_kA, wmk_t, slAr)
            cast_kA = nc.vector.tensor_copy(out=sk_t[:, 0:272], in_=ps_kA)

            gateB = nc.tensor.ldweights(weights=fpad[:, 2316:2317].bitcast(bf16))
            gateB2 = nc.tensor.ldweights(weights=fpad[:, 3473:3474].bitcast(bf16))
            ps_qB = ps.tile([88, 264], f32, tag="A")
            ps_kB = ps.tile([88, 264], f32, tag="B")
            qb = s1(ps_qB, wmq_t, slBr)
            _dep(qb, gateB)
            _dep(qb, gateB2)
            fb2 = fbar_part(2)
            fb3 = fbar_part(3)
            cast_qB = nc.vector.tensor_copy(out=sq_t[:, 272:536], in_=ps_qB)
            kb = s1(ps_kB, wmk_t, slBr)
            cast_kB = nc.vector.tensor_copy(out=sk_t[:, 272:536], in_=ps_kB)

            # --- stage 2: k then q, separate base-0 PSUM accumulators
            sq3 = sq_t.rearrange("p (rr ox) -> p rr ox", ox=8)
            sk3 = sk_t.rearrange("p (rr ox) -> p rr ox", ox=8)
            # q group first: its cast lands earlier, so PE streams stage-2-q
            # while the k-conv's last PSUM cast is still finishing on DVE
            psc_k = ps.tile([8, 64], f32, tag="D")
            psc_q = ps.tile([8, 64], f32, tag="C")
            for ky in range(11):
                nc.tensor.matmul(
                    psc_q, ws_t[:, ky * 8 : ky * 8 + 8],
                    sq3[:, ky : ky + 57 : 8, :],
                    start=(ky == 0), stop=(ky == 10),
                )
            nc.scalar.activation(
                out=q_t, in_=psc_q, func=AF.Gelu, bias=bq_v, scale=1.0
            )
            for ky in range(11):
                nc.tensor.matmul(
                    psc_k, ws_t[:, ky * 8 : ky * 8 + 8],
                    sk3[:, ky : ky + 57 : 8, :],
                    start=(ky == 0), stop=(ky == 10),
                )
            nc.scalar.activation(
                out=k_t, in_=psc_k, func=AF.Gelu, bias=bk_v, scale=1.0
            )

            # --- vbar path (fbar parts already reduced during stage 1)
            gate_v = nc.tensor.ldweights(weights=fbar_t[:, 0:1].bitcast(bf16))
            gate_v2 = nc.tensor.ldweights(weights=fbar_t[:, 63:64].bitcast(bf16))
            psv = ps.tile([64, 8], f32, tag="A")
            vmm = nc.tensor.matmul(
                psv, fbar_t, wvt_v, start=True, stop=True
            )
            _dep(vmm, gate_v)
            _dep(vmm, gate_v2)
            nc.scalar.copy(out=vaug_t[:, 0:8], in_=psv)

            # --- dots_T[J,I] = sum_c k[c,J] q[c,I];  e = exp(scale * dots_T)
            gate2 = nc.tensor.ldweights(weights=k_t[:, 0:1].bitcast(bf16))
            psd = ps.tile([64, 64], f32, tag="B")
            dmm = nc.tensor.matmul(psd, k_t, q_t, start=True, stop=True)
            _dep(dmm, gate2)
            nc.scalar.activation(out=e_t, in_=psd, func=AF.Exp, scale=SCALE)

            # --- out_u[I, 0:8] = sum_J e[J,I] vbar[J,c]; col 8 = 64*sum_J e
            gate_o = nc.tensor.ldweights(weights=e_t[:, 0:1].bitcast(bf16))
            pso = ps.tile([64, 9], f32, tag="A")
            omm = nc.tensor.matmul(pso, e_t, vaug_t, start=True, stop=True)
            _dep(omm, gate_o)
            nc.vector.reciprocal(out=rs_t, in_=pso[:, 8:9])
            nc.vector.tensor_scalar_mul(olT_t, pso[:, 0:8], rs_t)

            # --- broadcast along y: single DVE copy with stride-0 read on y
            import concourse.bass as bass
            T3 = T_t.rearrange("p (c y) -> p c y", y=64)
            ola = olT_t[:]
            ol_b = bass.AP(
                tensor=ola.tensor, offset=ola.offset,
                ap=[list(ola.ap[0]), list(ola.ap[1]), [0, 64]],
            )
            nc.vector.tensor_copy(out=T3, in_=ol_b)

            # --- store: out[c, x, y] <- T[x, c, y]
            out_ap = out_d[:].rearrange("c (x y) -> c x y", y=64).transpose([1, 0, 2])
            nc.sync.dma_start(out=out_ap, in_=T3)

    nc.finalize()
    return nc


def _get_nc():
    if "nc" not in _CACHE:
        _CACHE["nc"] = _build_nc()
    return _CACHE["nc"]


_WSEL = np.eye(88, dtype=np.float32)


def kernel(**inputs):
    global LAST_RESULTS
    from concourse.bass_utils import run_bass_kernel_spmd

    f = np.ascontiguousarray(inputs["f"], np.float32)
    w_qkv = np.ascontiguousarray(inputs["w_qkv"], np.float32)[:, :, 0, 0]  # [192,64]
    wq = np.ascontiguousarray(inputs["wq"], np.float32)
    wk = np.ascontiguousarray(inputs["wk"], np.float32)
    bq = np.ascontiguousarray(inputs["bq"], np.float32)
    bk = np.ascontiguousarray(inputs["bk"], np.float32)

    f2 = np.zeros((64, 68, 68), np.float32)
    f2[:, 2:66, 2:66] = f[0]
    f2 = f2.reshape(64, 68 * 68)

    in_maps = []
    for i in range(N_CORES):
        sl = slice(8 * i, 8 * i + 8)
        wp = np.zeros((64, 2064), np.float16)
        # [oc,ic,ky,kx] slice -> [ic,kx,ky,oc]
        wp[:, C_WQR:C_W1Q] = wq[sl].transpose(1, 3, 2, 0).reshape(64, 968)
        wp[:, C_W1Q:C_WKR] = w_qkv[0:64]
        wp[:, C_WKR:C_W1K] = wk[sl].transpose(1, 3, 2, 0).reshape(64, 968)
        wp[:, C_W1K:C_WVT] = w_qkv[64:128]
        w2 = np.zeros((64, 16), np.float32)
        w2[:, 0:8] = w_qkv[128 + 8 * i : 136 + 8 * i].T
        w2[0:8, 8] = bq[sl]
        w2[0:8, 9] = bk[sl]
        in_maps.append({"f": f2, "wp": wp, "w2": w2, "ws": _WSEL})

    nc = _get_nc()
    res = run_bass_kernel_spmd(nc, in_maps, core_ids=list(range(N_CORES)))
    LAST_RESULTS = res
    out = np.concatenate([r["out"] for r in res.results], axis=0)  # [64, 4096]
    return out.reshape(1, 64, 64, 64)



# revision 5
# speedup vs baseline: 1.0308x; 1.0308x over previous
"""Trainium2 Bass kernel for nn_Attention_32049045963483 (sparse_attention).

Math collapse (validated vs reference at ~2.7e-4 l2 rel err):
  - qkv 1x1 conv folds into the 11x11/stride-8 down-convs HOST-SIDE:
      conv(W1 @ f, wq) == conv(f, w_eff),  w_eff[d, kx, ky, oc] = sum_ic
      wq[oc,ic,ky,kx] W1[ic,d]  (weight preprocessing, like BN folding)
  - nearest-neighbor 64x upsample of the [64,64] score map + softmax over the
    upsampled axis == softmax of the low-res map; every output row depends
    only on x = i//64.
  - v enters only through 64-wide block sums: vbar = Wv @ fbar,
      fbar[d,J] = sum_y f[d,J,y]  (v never materializes)
  - out[c,x,y] = (sum_J e[J,x] vbar[J,c]) / (64 sum_J e[J,x]), broadcast on y.
  - exp via the tanh table (exact identity): e^x = 2/(1 - tanh(x/2)) - 1.
    tanh lives in the SAME ACT table as gelu, so one table load serves the
    whole kernel (no 1.3us reload on the critical path).  Safe here:
    scale*dots in [-0.5, 2.7].

Sharding: head-parallel over 8 cores; core i takes head i (out channels
8i..8i+7).  Each core reads full f (the down-convs mix all 64 channels).

Stage 1 packs TWO kx taps per matmul into the full 128-partition contraction
(f2 rows 64..127 hold f shifted by one column, prepared host-side), halving
PE streaming time vs the 64-deep version.  All conv operands fp16 (1 cyc/col
on PE, half the DMA bytes of fp32).
  stage 1: s[(ky,oc), r, cv, ox] = sum_{d,2taps} wE . f2, 2 r-chunks
           (rows 0..33 then 34..66) x 2 convs x 6 kx-pairs = 24 matmuls
  stage 2: psc[oc, (oy,cv,ox)] = sum_ky s[(ky,oc), ky+8oy, cv, ox] via
           identity-slice stationaries; 11 matmuls of 128 cols (q+k fused).
"""

import numpy as np

N_CORES = 8
SCALE = 8.0 ** -0.5  # dim_head ** -0.5

_CACHE = {}

LAST_RESULTS = None  # BassKernelResults of the most recent run (for test harness)


def _dep(after, before, sync=False):
    from concourse.tile import add_dep_helper

    a = getattr(after, "ins", after)
    b = getattr(before, "ins", before)
    add_dep_helper(a, b, sync=sync, reason="pin order")


def _build_nc():
    from contextlib import ExitStack

    import concourse.bacc as bacc
    import concourse.bass as bass
    import concourse.mybir as mybir
    import concourse.tile as tile

    f32 = mybir.dt.float32
    f16 = mybir.dt.float16
    X = mybir.AxisListType.X
    AF = mybir.ActivationFunctionType
    ALU = mybir.AluOpType

    nc = bacc.Bacc("TRN2", target_bir_lowering=False)

    f2_d = nc.dram_tensor("f2", [128, 67 * 68], f16, kind="ExternalInput")
    we_d = nc.dram_tensor("we", [128, 12 * 88], f16, kind="ExternalInput")
    ws_d = nc.dram_tensor("ws", [88, 88], f16, kind="ExternalInput")
    w2_d = nc.dram_tensor("w2", [64, 16], f32, kind="ExternalInput")
    out_d = nc.dram_tensor("out", [8, 4096], f32, kind="ExternalOutput")

    with tile.TileContext(nc) as tc:
        with ExitStack() as ctx:
            sb = ctx.enter_context(tc.tile_pool(name="sb", bufs=1))
            ps = ctx.enter_context(tc.tile_pool(name="ps", bufs=1, space="PSUM"))

            f2_t = sb.tile([128, 67 * 68], f16)
            we_t = sb.tile([128, 12 * 88], f16)
            ws_t = sb.tile([88, 88], f16)
            w2_t = sb.tile([64, 16], f32)
            s_t = sb.tile([88, 67 * 16], f16)
            qk_t = sb.tile([8, 128], f32)
            th_t = sb.tile([64, 64], f32)
            e_t = sb.tile([64, 64], f32)
            fbar_t = sb.tile([64, 64], f32)
            vaug_t = sb.tile([64, 9], f32)
            rs_t = sb.tile([64, 1], f32)
            T_t = sb.tile([64, 8 * 64], f32)
            scr_t = sb.tile([1, 1], f32)
            scr2_t = sb.tile([1, 1], f32)

            # --- input DMAs: partition-split across the two HWDGE rings so
            # every packet stays a full row (bigger descriptors).  Per-ring
            # FIFO order = priority: wE-q, f2 rows 0..33, wE-k, f2 rows
            # 34..66, then the small ws/w2.
            CA = 34 * 68  # f2 columns of r-chunk A
            d_weq0 = nc.sync.dma_start(out=we_t[0:64, 0:528], in_=we_d[0:64, 0:528])
            d_weq1 = nc.scalar.dma_start(out=we_t[64:128, 0:528], in_=we_d[64:128, 0:528])
            d_fA0 = nc.sync.dma_start(out=f2_t[0:64, 0:CA], in_=f2_d[0:64, 0:CA])
            d_fA1 = nc.scalar.dma_start(out=f2_t[64:128, 0:CA], in_=f2_d[64:128, 0:CA])
            d_wek0 = nc.sync.dma_start(out=we_t[0:64, 528:1056], in_=we_d[0:64, 528:1056])
            d_wek1 = nc.scalar.dma_start(out=we_t[64:128, 528:1056], in_=we_d[64:128, 528:1056])
            d_fB0 = nc.sync.dma_start(out=f2_t[0:64, CA:4556], in_=f2_d[0:64, CA:4556])
            d_fB1 = nc.scalar.dma_start(out=f2_t[64:128, CA:4556], in_=f2_d[64:128, CA:4556])
            d_ws = nc.sync.dma_start(out=ws_t, in_=ws_d[:])
            d_w2 = nc.sync.dma_start(out=w2_t, in_=w2_d[:])

            # constants + ACT warm-up: the dummy Gelu forces the single
            # gelu_and_others table load early (hidden behind the DMA wait);
            # tanh shares that table so no reload ever happens.
            nc.vector.memset(scr_t, 0.0)
            nc.vector.memset(vaug_t[:, 8:9], 64.0)
            dg = nc.scalar.activation(out=scr2_t, in_=scr_t, func=AF.Gelu)
            _dep(dg, d_fB1)

            f23 = f2_t.rearrange("p (r c) -> p r c", c=68)
            s3 = s_t.rearrange("p (r c16) -> p r c16", c16=16)
            bq_v = w2_t[0:8, 8:9]
            bk_v = w2_t[0:8, 9:10]
            wv_v = w2_t[:, 0:8]

            # --- stage 1: 2 r-chunks x 2 convs x 6 kx-pairs, fp16, 128-deep
            ps_Aq = ps.tile([88, 34 * 8], f32, tag="A")
            ps_Ak = ps.tile([88, 34 * 8], f32, tag="B")
            ps_Bq = ps.tile([88, 33 * 8], f32, tag="C")
            ps_Bk = ps.tile([88, 33 * 8], f32, tag="D")
            ps1 = {
                ("A", 0): ps_Aq,
                ("A", 1): ps_Ak,
                ("B", 0): ps_Bq,
                ("B", 1): ps_Bk,
            }
            chunks = {"A": slice(0, 34), "B": slice(34, 67)}

            def s1(ck, cv):
                pst = ps1[(ck, cv)]
                for g in range(6):
                    nc.tensor.matmul(
                        pst,
                        we_t[:, (6 * cv + g) * 88 : (6 * cv + g) * 88 + 88],
                        f23[:, chunks[ck], 2 * g : 2 * g + 57 : 8],
                        start=(g == 0),
                        stop=(g == 5),
                    )

            def s1cast(ck, cv):
                rsl = chunks[ck]
                n = rsl.stop - rsl.start
                pin = ps1[(ck, cv)].rearrange("p (r ox) -> p r ox", ox=8)
                nc.vector.tensor_copy(
                    out=s3[:, rsl, 8 * cv : 8 * cv + 8], in_=pin
                )

            # fbar sub-reduces on DVE from fp16 f2 rows (2x 16-bit rate)
            s1("A", 0)
            fbA = nc.vector.reduce_sum(
                out=fbar_t[:, 0:32], in_=f23[0:64, 2:34, 2:66], axis=X
            )
            s1cast("A", 0)
            s1("A", 1)
            s1cast("A", 1)
            s1("B", 0)
            fbB = nc.vector.reduce_sum(
                out=fbar_t[:, 32:64], in_=f23[0:64, 34:66, 2:66], axis=X
            )
            s1cast("B", 0)
            s1("B", 1)
            s1cast("B", 1)

            # --- stage 2: 11 ky-selection matmuls, q+k fused (128 cols each)
            psc = ps.tile([8, 128], f32, tag="E")
            sflat = s_t[:]
            for ky in range(11):
                a = s_t[:, ky * 16 : 1072]
                mv = bass.AP(
                    tensor=a.tensor,
                    offset=a.offset,
                    ap=[list(a.ap[0]), [128, 8], [1, 16]],
                )
                nc.tensor.matmul(
                    psc,
                    ws_t[:, ky * 8 : ky * 8 + 8],
                    mv,
                    start=(ky == 0),
                    stop=(ky == 10),
                )

            # --- vbar: psv[J, c] = fbar^T @ Wv^T; vaug col 8 = 64.0
            psv = ps.tile([64, 8], f32, tag="G")
            nc.tensor.matmul(psv, fbar_t, wv_v, start=True, stop=True)
            nc.vector.tensor_copy(out=vaug_t[:, 0:8], in_=psv)

            # --- gelu (exact, table) on q and k halves of psc
            qk3 = qk_t.rearrange("p (b x) -> p b x", x=8)
            aq = psc[:, 0:128]
            gin_q = bass.AP(
                tensor=aq.tensor, offset=aq.offset,
                ap=[list(aq.ap[0]), [16, 8], [1, 8]],
            )
            ak = psc[:, 8:128]
            gin_k = bass.AP(
                tensor=ak.tensor, offset=ak.offset,
                ap=[list(ak.ap[0]), [16, 8], [1, 8]],
            )
            nc.scalar.activation(
                out=qk3[:, 0:8, :], in_=gin_q, func=AF.Gelu, bias=bq_v, scale=1.0
            )
            nc.scalar.activation(
                out=qk3[:, 8:16, :], in_=gin_k, func=AF.Gelu, bias=bk_v, scale=1.0
            )

            # --- dots^T[J, I] then e = exp(scale*dots) via tanh identity
            psd = ps.tile([64, 64], f32, tag="F")
            nc.tensor.matmul(
                psd, qk_t[:, 64:128], qk_t[:, 0:64], start=True, stop=True
            )
            nc.scalar.activation(
                out=th_t, in_=psd, func=AF.Tanh, scale=SCALE * 0.5
            )
            nc.vector.tensor_scalar(
                out=e_t, in0=th_t, scalar1=-1.0, scalar2=1.0,
                op0=ALU.mult, op1=ALU.add,
            )
            nc.vector.reciprocal(out=th_t, in_=e_t)
            nc.vector.tensor_scalar(
                out=e_t, in0=th_t, scalar1=2.0, scalar2=1.0,
                op0=ALU.mult, op1=ALU.subtract,
            )

            # --- out_u[I, 0:8] = sum_J e[J,I] vaug[J,:]; col 8 = 64*sum_J e
            pso = ps.tile([64, 9], f32, tag="H")
            nc.tensor.matmul(pso, e_t, vaug_t, start=True, stop=True)
            nc.vector.reciprocal(out=rs_t, in_=pso[:, 8:9])

            # --- normalize + broadcast along y in ONE DVE op (stride-0 in)
            T3 = T_t.rearrange("p (c y) -> p c y", y=64)
            ao = pso[:, 0:8]
            o_b = bass.AP(
                tensor=ao.tensor, offset=ao.offset,
                ap=[list(ao.ap[0]), [1, 8], [0, 64]],
            )
            nc.vector.tensor_scalar_mul(T3, o_b, rs_t)

            # --- store out[c, x, y] <- T[x, c, y], split across both rings
            out_ap = out_d[:].rearrange("c (x y) -> c x y", y=64).transpose([1, 0, 2])
            nc.sync.dma_start(out=out_ap[0:32], in_=T3[0:32])
            nc.scalar.dma_start(out=out_ap[32:64], in_=T3[32:64])

    nc.finalize()
    return nc


def _get_nc():
    if "nc" not in _CACHE:
        _CACHE["nc"] = _build_nc()
    return _CACHE["nc"]


_WSEL = np.eye(88, dtype=np.float16)


def kernel(**inputs):
    global LAST_RESULTS
    from concourse.bass_utils import run_bass_kernel_spmd

    f = np.ascontiguousarray(inputs["f"], np.float32)
    w_qkv = np.ascontiguousarray(inputs["w_qkv"], np.float32)[:, :, 0, 0]  # [192,64]
    wq = np.ascontiguousarray(inputs["wq"], np.float32)
    wk = np.ascontiguousarray(inputs["wk"], np.float32)
    bq = np.ascontiguousarray(inputs["bq"], np.float32)
    bk = np.ascontiguousarray(inputs["bk"], np.float32)

    W1q, W1k, Wv = w_qkv[0:64], w_qkv[64:128], w_qkv[128:192]

    # f2: rows 0..63 = padded f, rows 64..127 = same shifted one column (+x
    # tap) so stage 1 contracts over 128 partitions = 64 ch x 2 kx taps.
    fpad = np.zeros((64, 68, 68), np.float32)
    fpad[:, 2:66, 2:66] = f[0]
    f2 = np.zeros((128, 67, 68), np.float32)
    f2[0:64] = fpad[:, 0:67, :]
    f2[64:128, :, 0:67] = fpad[:, 0:67, 1:68]
    f2 = f2.reshape(128, 67 * 68).astype(np.float16)

    in_maps = []
    for i in range(N_CORES):
        sl = slice(8 * i, 8 * i + 8)
        # w_eff[d, kx, ky, oc] = sum_ic wq[oc,ic,ky,kx] W1[ic,d]
        wEq = np.einsum("oiyx,id->dxyo", wq[sl], W1q)
        wEk = np.einsum("oiyx,id->dxyo", wk[sl], W1k)
        wE = np.zeros((128, 12, 88), np.float16)
        for g in range(6):
            wE[0:64, g] = wEq[:, 2 * g].reshape(64, 88)
            wE[0:64, 6 + g] = wEk[:, 2 * g].reshape(64, 88)
            if 2 * g + 1 <= 10:
                wE[64:128, g] = wEq[:, 2 * g + 1].reshape(64, 88)
                wE[64:128, 6 + g] = wEk[:, 2 * g + 1].reshape(64, 88)
        w2 = np.zeros((64, 16), np.float32)
        w2[:, 0:8] = Wv[sl].T
        w2[0:8, 8] = bq[sl]
        w2[0:8, 9] = bk[sl]
        in_maps.append(
            {"f2": f2, "we": wE.reshape(128, 1056), "ws": _WSEL, "w2": w2}
        )

    nc = _get_nc()
    res = run_bass_kernel_spmd(nc, in_maps, core_ids=list(range(N_CORES)))
    LAST_RESULTS = res
    out = np.concatenate([r["out"] for r in res.results], axis=0)  # [64, 4096]
    return out.reshape(1, 64, 64, 64)


# revision 6
# speedup vs baseline: 1.1739x; 1.1388x over previous
"""Trainium2 Bass kernel for nn_Attention_32049045963483 (sparse_attention).

Math collapse (validated vs reference at ~3e-4 l2 rel err):
  - qkv 1x1 conv folds into the 11x11/stride-8 down-convs HOST-SIDE:
      conv(W1 @ f, wq) == conv(f, w_eff),  w_eff[d, kx, ky, oc] = sum_ic
      wq[oc,ic,ky,kx] W1[ic,d]  (weight preprocessing, like BN folding)
  - nearest-neighbor 64x upsample of the [64,64] score map + softmax over the
    upsampled axis == softmax of the low-res map; every output row depends
    only on x = i//64.
  - v enters only through 64-wide block sums: vbar = Wv @ fbar,
      fbar[d,J] = sum_y f[d,J,y]  (v never materializes)
  - out[c,x,y] = (sum_J e[J,x] vbar[J,c]) / (64 sum_J e[J,x]), broadcast on y.

Sharding: head-parallel over 8 cores; core i takes head i (out channels
8i..8i+7).  Each core reads full f (the down-convs mix all 64 channels).

Stage 1 packs TWO kx taps per matmul into the full 128-partition contraction.
f2E is a phase-major permutation of the padded f: rows 0..63 hold the EVEN
columns (c = 2m) laid out as [r, m%4, m//4], rows 64..127 the ODD columns
(the +1-shifted tap).  This (a) gives every matmul a contiguous 8-element
inner dim (fp16 runs at full 1 col/cycle only with a contiguous inner dim;
strided-8 fp16 measured 2x slower), and (b) stores each f element exactly
once -- no duplication, so f DMA halves vs fp32.
  stage 1: 2 r-chunks x 2 convs x 6 kx-pairs = 24 matmuls, fp16
  stage 2: psc[oc, (oy,cv,ox)] = sum_ky s[(ky,oc), ky+8oy, cv, ox] via
           identity-slice stationaries; 11 matmuls of 128 cols (q+k fused).
  fbar[d,J] = sum(f2E[d, J+2, :]) + sum(f2E[64+d, J+2, :])  (pads are 0).
Softmax exp runs on the ACT engine; both needed tables (gelu + exp) are
preloaded into the two table slots early (hidden behind the input DMA), so
no mid-kernel table reload (measured: the load pass hoists both).
"""

import numpy as np

N_CORES = 8
SCALE = 8.0 ** -0.5  # dim_head ** -0.5

_CACHE = {}

LAST_RESULTS = None  # BassKernelResults of the most recent run (for test harness)


def _dep(after, before, sync=False):
    from concourse.tile import add_dep_helper

    a = getattr(after, "ins", after)
    b = getattr(before, "ins", before)
    add_dep_helper(a, b, sync=sync, reason="pin order")


def _build_nc():
    from contextlib import ExitStack

    import concourse.bacc as bacc
    import concourse.bass as bass
    import concourse.mybir as mybir
    import concourse.tile as tile

    f32 = mybir.dt.float32
    f16 = mybir.dt.float16
    X = mybir.AxisListType.X
    AF = mybir.ActivationFunctionType
    ALU = mybir.AluOpType

    nc = bacc.Bacc("TRN2", target_bir_lowering=False)

    f2_d = nc.dram_tensor("f2", [128, 67 * 36], f16, kind="ExternalInput")
    we_d = nc.dram_tensor("we", [128, 12 * 88], f16, kind="ExternalInput")
    ws_d = nc.dram_tensor("ws", [88, 88], f16, kind="ExternalInput")
    w2_d = nc.dram_tensor("w2", [64, 16], f32, kind="ExternalInput")
    out_d = nc.dram_tensor("out", [8, 4096], f32, kind="ExternalOutput")

    with tile.TileContext(nc) as tc:
        with ExitStack() as ctx:
            sb = ctx.enter_context(tc.tile_pool(name="sb", bufs=1))
            ps = ctx.enter_context(tc.tile_pool(name="ps", bufs=1, space="PSUM"))

            f2_t = sb.tile([128, 67 * 36], f16)
            we_t = sb.tile([128, 12 * 88], f16)
            ws_t = sb.tile([88, 88], f16)
            w2_t = sb.tile([64, 16], f32)
            s_t = sb.tile([88, 67 * 16], f16)
            qk_t = sb.tile([8, 128], f16)
            e_t = sb.tile([64, 64], f16)
            fbL_t = sb.tile([64, 64], f32)
            fbU_t = sb.tile([64, 64], f32)
            fbar_t = sb.tile([64, 64], f32)
            vaug_t = sb.tile([64, 9], f16)
            rs_t = sb.tile([64, 1], f32)
            T_t = sb.tile([64, 8 * 64], f32)
            scr_t = sb.tile([1, 1], f32)
            scr2_t = sb.tile([1, 1], f32)

            # --- input DMAs: partition-split across the two HWDGE rings so
            # every packet stays a full row.  Per-ring FIFO order = priority:
            # wE-q, f2 rows 0..33, wE-k, f2 rows 34..66, then small ws/w2.
            CA = 34 * 36  # f2E columns of r-chunk A
            CT = 67 * 36
            d_weq0 = nc.sync.dma_start(out=we_t[0:64, 0:528], in_=we_d[0:64, 0:528])
            d_weq1 = nc.scalar.dma_start(out=we_t[64:128, 0:528], in_=we_d[64:128, 0:528])
            d_fA0 = nc.sync.dma_start(out=f2_t[0:64, 0:CA], in_=f2_d[0:64, 0:CA])
            d_fA1 = nc.scalar.dma_start(out=f2_t[64:128, 0:CA], in_=f2_d[64:128, 0:CA])
            d_wek0 = nc.sync.dma_start(out=we_t[0:64, 528:1056], in_=we_d[0:64, 528:1056])
            d_wek1 = nc.scalar.dma_start(out=we_t[64:128, 528:1056], in_=we_d[64:128, 528:1056])
            d_fB0 = nc.sync.dma_start(out=f2_t[0:64, CA:CT], in_=f2_d[0:64, CA:CT])
            d_fB1 = nc.scalar.dma_start(out=f2_t[64:128, CA:CT], in_=f2_d[64:128, CA:CT])
            d_ws = nc.sync.dma_start(out=ws_t, in_=ws_d[:])
            d_w2 = nc.sync.dma_start(out=w2_t, in_=w2_d[:])

            # constants + ACT warm-up: dummy Gelu then dummy Exp force BOTH
            # table slots to load early (hidden behind the DMA wait).
            nc.vector.memset(scr_t, 0.0)
            nc.vector.memset(vaug_t[:, 8:9], 64.0)
            dg = nc.scalar.activation(out=scr2_t, in_=scr_t, func=AF.Gelu)
            de = nc.scalar.activation(out=scr2_t, in_=scr_t, func=AF.Exp)
            _dep(dg, d_fB1)
            _dep(de, dg)

            f23 = f2_t.rearrange("p (r s) -> p r s", s=36)
            s3 = s_t.rearrange("p (r c16) -> p r c16", c16=16)
            bq_v = w2_t[0:8, 8:9]
            bk_v = w2_t[0:8, 9:10]
            wv_v = w2_t[:, 0:8]

            # --- stage 1: 2 r-chunks x 2 convs x 6 kx-pairs, fp16, 128-deep
            ps_Aq = ps.tile([88, 34 * 8], f32, tag="A")
            ps_Ak = ps.tile([88, 34 * 8], f32, tag="B")
            ps_Bq = ps.tile([88, 33 * 8], f32, tag="C")
            ps_Bk = ps.tile([88, 33 * 8], f32, tag="D")
            ps1 = {
                ("A", 0): ps_Aq,
                ("A", 1): ps_Ak,
                ("B", 0): ps_Bq,
                ("B", 1): ps_Bk,
            }
            chunks = {"A": slice(0, 34), "B": slice(34, 67)}

            def s1(ck, cv):
                pst = ps1[(ck, cv)]
                for g in range(6):
                    base = (g % 4) * 9 + (g // 4)
                    nc.tensor.matmul(
                        pst,
                        we_t[:, (6 * cv + g) * 88 : (6 * cv + g) * 88 + 88],
                        f23[:, chunks[ck], base : base + 8],
                        start=(g == 0),
                        stop=(g == 5),
                    )

            def s1cast(ck, cv):
                rsl = chunks[ck]
                pin = ps1[(ck, cv)].rearrange("p (r ox) -> p r ox", ox=8)
                nc.vector.tensor_copy(out=s3[:, rsl, 8 * cv : 8 * cv + 8], in_=pin)

            # fbar: sum ALL 36 slots per row (pads are zero) for both halves.
            # Emitted in DVE-queue order: fbar parts run while stage-1 PSUMs
            # are still accumulating, casts after each chunk completes.
            s1("A", 0)
            nc.vector.reduce_sum(out=fbL_t[:, 0:32], in_=f23[0:64, 2:34, :], axis=X)
            nc.vector.reduce_sum(out=fbU_t[:, 0:32], in_=f23[64:128, 2:34, :], axis=X)
            nc.vector.tensor_tensor(
                out=fbar_t[:, 0:32], in0=fbL_t[:, 0:32], in1=fbU_t[:, 0:32],
                op=ALU.add,
            )
            s1cast("A", 0)
            s1("A", 1)
            s1cast("A", 1)
            s1("B", 0)
            nc.vector.reduce_sum(out=fbL_t[:, 32:64], in_=f23[0:64, 34:66, :], axis=X)
            nc.vector.reduce_sum(out=fbU_t[:, 32:64], in_=f23[64:128, 34:66, :], axis=X)
            nc.vector.tensor_tensor(
                out=fbar_t[:, 32:64], in0=fbL_t[:, 32:64], in1=fbU_t[:, 32:64],
                op=ALU.add,
            )
            s1cast("B", 0)
            s1("B", 1)
            s1cast("B", 1)

            # --- stage 2: 11 ky-selection matmuls, q+k fused (128 cols each)
            psc = ps.tile([8, 128], f32, tag="E")
            for ky in range(11):
                a = s_t[:, ky * 16 : 1072]
                mv = bass.AP(
                    tensor=a.tensor,
                    offset=a.offset,
                    ap=[list(a.ap[0]), [128, 8], [1, 16]],
                )
                nc.tensor.matmul(
                    psc,
                    ws_t[:, ky * 8 : ky * 8 + 8],
                    mv,
                    start=(ky == 0),
                    stop=(ky == 10),
                )

            # --- vbar: psv[J, c] = fbar^T @ Wv^T; vaug col 8 = 64.0
            psv = ps.tile([64, 8], f32, tag="G")
            nc.tensor.matmul(psv, fbar_t, wv_v, start=True, stop=True)
            nc.vector.tensor_copy(out=vaug_t[:, 0:8], in_=psv)

            # --- gelu (exact, table) on q and k halves of psc -> fp16 qk
            qk3 = qk_t.rearrange("p (b x) -> p b x", x=8)
            aq = psc[:, 0:128]
            gin_q = bass.AP(
                tensor=aq.tensor, offset=aq.offset,
                ap=[list(aq.ap[0]), [16, 8], [1, 8]],
            )
            ak = psc[:, 8:128]
            gin_k = bass.AP(
                tensor=ak.tensor, offset=ak.offset,
                ap=[list(ak.ap[0]), [16, 8], [1, 8]],
            )
            nc.scalar.activation(
                out=qk3[:, 0:8, :], in_=gin_q, func=AF.Gelu, bias=bq_v, scale=1.0
            )
            nc.scalar.activation(
                out=qk3[:, 8:16, :], in_=gin_k, func=AF.Gelu, bias=bk_v, scale=1.0
            )

            # --- dots^T[J, I] (fp16, single-pass) then e = exp(scale*dots)
            psd = ps.tile([64, 64], f32, tag="F")
            nc.tensor.matmul(
                psd, qk_t[:, 64:128], qk_t[:, 0:64], start=True, stop=True
            )
            nc.scalar.activation(out=e_t, in_=psd, func=AF.Exp, scale=SCALE)

            # --- out_u[I, 0:8] = sum_J e[J,I] vaug[J,:]; col 8 = 64*sum_J e
            pso = ps.tile([64, 9], f32, tag="H")
            nc.tensor.matmul(pso, e_t, vaug_t, start=True, stop=True)
            nc.vector.reciprocal(out=rs_t, in_=pso[:, 8:9])

            # --- normalize + broadcast along y in ONE DVE op (stride-0 in)
            T3 = T_t.rearrange("p (c y) -> p c y", y=64)
            ao = pso[:, 0:8]
            o_b = bass.AP(
                tensor=ao.tensor, offset=ao.offset,
                ap=[list(ao.ap[0]), [1, 8], [0, 64]],
            )
            nc.vector.tensor_scalar_mul(T3, o_b, rs_t)

            # --- store out[c, x, y] <- T[x, c, y], split across both rings
            out_ap = out_d[:].rearrange("c (x y) -> c x y", y=64).transpose([1, 0, 2])
            nc.sync.dma_start(out=out_ap[0:32], in_=T3[0:32])
            nc.scalar.dma_start(out=out_ap[32:64], in_=T3[32:64])

    nc.finalize()
    return nc


def _get_nc():
    if "nc" not in _CACHE:
        _CACHE["nc"] = _build_nc()
    return _CACHE["nc"]


_WSEL = np.eye(88, dtype=np.float16)


def kernel(**inputs):
    global LAST_RESULTS
    from concourse.bass_utils import run_bass_kernel_spmd

    f = np.ascontiguousarray(inputs["f"], np.float32)
    w_qkv = np.ascontiguousarray(inputs["w_qkv"], np.float32)[:, :, 0, 0]  # [192,64]
    wq = np.ascontiguousarray(inputs["wq"], np.float32)
    wk = np.ascontiguousarray(inputs["wk"], np.float32)
    bq = np.ascontiguousarray(inputs["bq"], np.float32)
    bk = np.ascontiguousarray(inputs["bk"], np.float32)

    W1q, W1k, Wv = w_qkv[0:64], w_qkv[64:128], w_qkv[128:192]

    # f2E phase-major permutation: slot s = (m%4)*9 + m//4 holds column 2m
    # (rows 0..63) / column 2m+1 (rows 64..127) of the padded f.
    fpad = np.zeros((64, 68, 68), np.float32)
    fpad[:, 2:66, 2:66] = f[0]
    f2 = np.zeros((128, 67, 36), np.float32)
    for m in range(34):
        s = (m % 4) * 9 + m // 4
        f2[0:64, :, s] = fpad[:, 0:67, 2 * m]
        if 2 * m + 1 <= 67:
            f2[64:128, :, s] = fpad[:, 0:67, 2 * m + 1]
    f2 = f2.reshape(128, 67 * 36).astype(np.float16)

    in_maps = []
    for i in range(N_CORES):
        sl = slice(8 * i, 8 * i + 8)
        # w_eff[d, kx, ky, oc] = sum_ic wq[oc,ic,ky,kx] W1[ic,d]
        wEq = np.einsum("oiyx,id->dxyo", wq[sl], W1q)
        wEk = np.einsum("oiyx,id->dxyo", wk[sl], W1k)
        wE = np.zeros((128, 12, 88), np.float16)
        for g in range(6):
            wE[0:64, g] = wEq[:, 2 * g].reshape(64, 88)
            wE[0:64, 6 + g] = wEk[:, 2 * g].reshape(64, 88)
            if 2 * g + 1 <= 10:
                wE[64:128, g] = wEq[:, 2 * g + 1].reshape(64, 88)
                wE[64:128, 6 + g] = wEk[:, 2 * g + 1].reshape(64, 88)
        w2 = np.zeros((64, 16), np.float32)
        w2[:, 0:8] = Wv[sl].T
        w2[0:8, 8] = bq[sl]
        w2[0:8, 9] = bk[sl]
        in_maps.append(
            {"f2": f2, "we": wE.reshape(128, 1056), "ws": _WSEL, "w2": w2}
        )

    nc = _get_nc()
    res = run_bass_kernel_spmd(nc, in_maps, core_ids=list(range(N_CORES)))
    LAST_RESULTS = res
    out = np.concatenate([r["out"] for r in res.results], axis=0)  # [64, 4096]
    return out.reshape(1, 64, 64, 64)


# revision 7
# speedup vs baseline: 1.3086x; 1.1148x over previous
"""Trainium2 Bass kernel for nn_Attention_32049045963483 (sparse_attention).

Math collapse (validated vs reference at ~4e-4 l2 rel err):
  - qkv 1x1 conv folds into the 11x11/stride-8 down-convs HOST-SIDE:
      conv(W1 @ f, wq) == conv(f, w_eff)   (weight preprocessing)
  - 64x nearest upsample + softmax == softmax of the low-res score map;
    every output row depends only on x = i//64.
  - v enters only through 64-wide block sums: vbar = Wv @ fbar,
      fbar[d,J] = sum_y f[d,J,y]  (v never materializes)
  - out[c,x,y] = (sum_J e[J,x] vbar[J,c]) / (64 sum_J e[J,x]), broadcast on y.
  - exp via the tanh table (exact identity): e^x = 2/(1 - tanh(x/2)) - 1.
    tanh shares an ACT table with gelu's load set, so no mid-kernel 1.3us
    table reload (Exp would force one: the ACT table slots reload on every
    func-set switch).  Safe: scale*dots in [-0.5, 2.7].

Sharding: head-parallel over 8 cores; core i takes head i.  Each core reads
full f (the down-convs mix all 64 channels).

Stage 1 packs TWO kx taps per matmul into the full 128-partition contraction.
f2E is a phase-major permutation of padded f: rows 0..63 hold EVEN columns
(c=2m) laid out as [r, m%4, m//4], rows 64..127 the ODD columns (the +1 tap).
This gives a contiguous 8-element inner dim (fp16 needs contiguity for
1 col/cycle; strided-8 fp16 measured 2x slower) and stores each element once.
fbar drops out as the full 36-slot row sums of BOTH partition halves (pads
are zero); Wv is stacked twice in the vbar contraction so no half-add needed.

All inputs ride in ONE dram blob with just 2 DMA pieces per HWDGE ring
(partition-split).  More pieces choke shared descriptor engine 79 (measured:
with 10 input triggers its per-piece share completed 2.5-3us after the other
15 engines, delaying every completion semaphore).
"""

import numpy as np

N_CORES = 8
SCALE = 8.0 ** -0.5  # dim_head ** -0.5

# blob column map (fp16 elements)
C_WEQ = 0
C_F2A = 528
C_WEK = 1752
C_F2B = 2280
C_WS = 3468
C_TOT = 3556

_CACHE = {}

LAST_RESULTS = None  # BassKernelResults of the most recent run (for test harness)


def _dep(after, before, sync=False):
    from concourse.tile import add_dep_helper

    a = getattr(after, "ins", after)
    b = getattr(before, "ins", before)
    add_dep_helper(a, b, sync=sync, reason="pin order")


def _build_nc():
    from contextlib import ExitStack

    import concourse.bacc as bacc
    import concourse.bass as bass
    import concourse.mybir as mybir
    import concourse.tile as tile

    f32 = mybir.dt.float32
    f16 = mybir.dt.float16
    X = mybir.AxisListType.X
    AF = mybir.ActivationFunctionType
    ALU = mybir.AluOpType

    nc = bacc.Bacc("TRN2", target_bir_lowering=False)

    blob_d = nc.dram_tensor("blob", [128, C_TOT], f16, kind="ExternalInput")
    w2_d = nc.dram_tensor("w2", [128, 16], f32, kind="ExternalInput")
    out_d = nc.dram_tensor("out", [8, 4096], f32, kind="ExternalOutput")

    with tile.TileContext(nc) as tc:
        with ExitStack() as ctx:
            sb = ctx.enter_context(tc.tile_pool(name="sb", bufs=1))
            ps = ctx.enter_context(tc.tile_pool(name="ps", bufs=1, space="PSUM"))

            blob_t = sb.tile([128, C_TOT], f16)
            w2_t = sb.tile([128, 16], f32)
            s_t = sb.tile([88, 67 * 16], f16)
            qk_t = sb.tile([8, 128], f16)
            th_t = sb.tile([64, 64], f32)
            u_t = sb.tile([64, 64], f32)
            e_t = sb.tile([64, 64], f16)
            fb_t = sb.tile([128, 64], f32)
            vaug_t = sb.tile([64, 9], f16)
            rs_t = sb.tile([64, 1], f32)
            T_t = sb.tile([64, 8 * 64], f32)
            scr_t = sb.tile([1, 1], f32)
            scr2_t = sb.tile([1, 1], f32)

            # --- input DMAs: 2 pieces per ring (partition-split), then w2
            d_p1s = nc.sync.dma_start(
                out=blob_t[0:64, 0:C_WEK], in_=blob_d[0:64, 0:C_WEK]
            )
            d_p1c = nc.scalar.dma_start(
                out=blob_t[64:128, 0:C_WEK], in_=blob_d[64:128, 0:C_WEK]
            )
            d_p2s = nc.sync.dma_start(
                out=blob_t[0:64, C_WEK:C_TOT], in_=blob_d[0:64, C_WEK:C_TOT]
            )
            d_p2c = nc.scalar.dma_start(
                out=blob_t[64:128, C_WEK:C_TOT], in_=blob_d[64:128, C_WEK:C_TOT]
            )
            d_w2 = nc.sync.dma_start(out=w2_t, in_=w2_d[:])

            # constants + ACT warm-up: dummy Gelu forces its table load early;
            # the load pass hoists tanh's set load next to it (measured).
            nc.vector.memset(scr_t, 0.0)
            nc.vector.memset(vaug_t[:, 8:9], 64.0)
            dg = nc.scalar.activation(out=scr2_t, in_=scr_t, func=AF.Gelu)
            dt = nc.scalar.activation(out=scr2_t, in_=scr_t, func=AF.Tanh)
            _dep(dg, d_p2c)
            _dep(dt, dg)

            f23A = blob_t[:, C_F2A:C_WEK].rearrange("p (r s) -> p r s", s=36)
            f23B = blob_t[:, C_F2B:C_WS].rearrange("p (r s) -> p r s", s=36)
            ws_v = blob_t[0:88, C_WS:C_TOT]
            s3 = s_t.rearrange("p (r c16) -> p r c16", c16=16)
            wv2_v = w2_t[:, 0:8]
            bq_v = w2_t[0:8, 8:9]
            bk_v = w2_t[0:8, 9:10]

            # --- stage 1: 2 r-chunks x 2 convs x 6 kx-pairs, fp16, 128-deep
            ps_Aq = ps.tile([88, 34 * 8], f32, tag="A")
            ps_Ak = ps.tile([88, 34 * 8], f32, tag="B")
            ps_Bq = ps.tile([88, 33 * 8], f32, tag="C")
            ps_Bk = ps.tile([88, 33 * 8], f32, tag="D")

            def s1(f23c, pst, wbase):
                for g in range(6):
                    base = (g % 4) * 9 + (g // 4)
                    nc.tensor.matmul(
                        pst,
                        blob_t[:, wbase + g * 88 : wbase + g * 88 + 88],
                        f23c[:, :, base : base + 8],
                        start=(g == 0),
                        stop=(g == 5),
                    )

            def s1cast(eng, pst, rsl, cv):
                pin = pst.rearrange("p (r ox) -> p r ox", ox=8)
                out = s3[:, rsl, 8 * cv : 8 * cv + 8]
                if eng == "v":
                    nc.vector.tensor_copy(out=out, in_=pin)
                else:
                    nc.scalar.copy(out=out, in_=pin)

            slA, slB = slice(0, 34), slice(34, 67)

            # DVE order: redA, castqA, redB, castqB, vaug, tanh-chain, out ops
            # ACT order: dummies, castkA, castkB, gelu q/k, tanh
            s1(f23A, ps_Aq, C_WEQ)
            nc.vector.reduce_sum(out=fb_t[:, 0:32], in_=f23A[:, 2:34, :], axis=X)
            s1cast("v", ps_Aq, slA, 0)
            s1(f23A, ps_Ak, C_WEK)
            s1cast("a", ps_Ak, slA, 1)
            s1(f23B, ps_Bq, C_WEQ)
            nc.vector.reduce_sum(out=fb_t[:, 32:64], in_=f23B[:, 0:32, :], axis=X)
            s1cast("v", ps_Bq, slB, 0)
            s1(f23B, ps_Bk, C_WEK)
            s1cast("a", ps_Bk, slB, 1)

            # --- stage 2: 11 ky-selection matmuls, q+k fused (128 cols each)
            psc = ps.tile([8, 128], f32, tag="E")
            for ky in range(11):
                a = s_t[:, ky * 16 : 1072]
                mv = bass.AP(
                    tensor=a.tensor,
                    offset=a.offset,
                    ap=[list(a.ap[0]), [128, 8], [1, 16]],
                )
                nc.tensor.matmul(
                    psc,
                    ws_v[:, ky * 8 : ky * 8 + 8],
                    mv,
                    start=(ky == 0),
                    stop=(ky == 10),
                )

            # --- vbar: psv[J, c] = sum over 128 rows of fb * [Wv.T; Wv.T]
            psv = ps.tile([64, 8], f32, tag="G")
            nc.tensor.matmul(psv, fb_t, wv2_v, start=True, stop=True)
            nc.vector.tensor_copy(out=vaug_t[:, 0:8], in_=psv)

            # --- gelu (exact, table) on q and k halves of psc -> fp16 qk
            qk3 = qk_t.rearrange("p (b x) -> p b x", x=8)
            aq = psc[:, 0:128]
            gin_q = bass.AP(
                tensor=aq.tensor, offset=aq.offset,
                ap=[list(aq.ap[0]), [16, 8], [1, 8]],
            )
            ak = psc[:, 8:128]
            gin_k = bass.AP(
                tensor=ak.tensor, offset=ak.offset,
                ap=[list(ak.ap[0]), [16, 8], [1, 8]],
            )
            nc.scalar.activation(
                out=qk3[:, 0:8, :], in_=gin_q, func=AF.Gelu, bias=bq_v, scale=1.0
            )
            nc.scalar.activation(
                out=qk3[:, 8:16, :], in_=gin_k, func=AF.Gelu, bias=bk_v, scale=1.0
            )

            # --- dots^T[J, I] (fp16, single-pass) then e via tanh identity
            psd = ps.tile([64, 64], f32, tag="F")
            nc.tensor.matmul(
                psd, qk_t[:, 64:128], qk_t[:, 0:64], start=True, stop=True
            )
            nc.scalar.activation(out=th_t, in_=psd, func=AF.Tanh, scale=SCALE * 0.5)
            nc.vector.tensor_scalar(
                out=u_t, in0=th_t, scalar1=-1.0, scalar2=1.0,
                op0=ALU.mult, op1=ALU.add,
            )
            nc.vector.reciprocal(out=th_t, in_=u_t)
            nc.vector.tensor_scalar(
                out=e_t, in0=th_t, scalar1=2.0, scalar2=1.0,
                op0=ALU.mult, op1=ALU.subtract,
            )

            # --- out_u[I, 0:8] = sum_J e[J,I] vaug[J,:]; col 8 = 64*sum_J e
            pso = ps.tile([64, 9], f32, tag="H")
            nc.tensor.matmul(pso, e_t, vaug_t, start=True, stop=True)
            nc.vector.reciprocal(out=rs_t, in_=pso[:, 8:9])

            # --- normalize + broadcast along y in ONE DVE op (stride-0 in)
            T3 = T_t.rearrange("p (c y) -> p c y", y=64)
            ao = pso[:, 0:8]
            o_b = bass.AP(
                tensor=ao.tensor, offset=ao.offset,
                ap=[list(ao.ap[0]), [1, 8], [0, 64]],
            )
            nc.vector.tensor_scalar_mul(T3, o_b, rs_t)

            # --- store out[c, x, y] <- T[x, c, y], split across both rings
            out_ap = out_d[:].rearrange("c (x y) -> c x y", y=64).transpose([1, 0, 2])
            nc.sync.dma_start(out=out_ap[0:32], in_=T3[0:32])
            nc.scalar.dma_start(out=out_ap[32:64], in_=T3[32:64])

    nc.finalize()
    return nc


def _get_nc():
    if "nc" not in _CACHE:
        _CACHE["nc"] = _build_nc()
    return _CACHE["nc"]


def kernel(**inputs):
    global LAST_RESULTS
    from concourse.bass_utils import run_bass_kernel_spmd

    f = np.ascontiguousarray(inputs["f"], np.float32)
    w_qkv = np.ascontiguousarray(inputs["w_qkv"], np.float32)[:, :, 0, 0]  # [192,64]
    wq = np.ascontiguousarray(inputs["wq"], np.float32)
    wk = np.ascontiguousarray(inputs["wk"], np.float32)
    bq = np.ascontiguousarray(inputs["bq"], np.float32)
    bk = np.ascontiguousarray(inputs["bk"], np.float32)

    W1q, W1k, Wv = w_qkv[0:64], w_qkv[64:128], w_qkv[128:192]

    # f2E phase-major permutation: slot s = (m%4)*9 + m//4 holds column 2m
    # (rows 0..63) / column 2m+1 (rows 64..127) of the padded f.
    fpad = np.zeros((64, 68, 68), np.float32)
    fpad[:, 2:66, 2:66] = f[0]
    f2 = np.zeros((128, 67, 36), np.float32)
    for m in range(34):
        s = (m % 4) * 9 + m // 4
        f2[0:64, :, s] = fpad[:, 0:67, 2 * m]
        if 2 * m + 1 <= 67:
            f2[64:128, :, s] = fpad[:, 0:67, 2 * m + 1]
    f2 = f2.astype(np.float16)

    eye88 = np.eye(88, dtype=np.float16)

    in_maps = []
    for i in range(N_CORES):
        sl = slice(8 * i, 8 * i + 8)
        # w_eff[d, kx, ky, oc] = sum_ic wq[oc,ic,ky,kx] W1[ic,d]
        wEq = np.einsum("oiyx,id->dxyo", wq[sl], W1q)
        wEk = np.einsum("oiyx,id->dxyo", wk[sl], W1k)
        wE = np.zeros((128, 12, 88), np.float16)
        for g in range(6):
            wE[0:64, g] = wEq[:, 2 * g].reshape(64, 88)
            wE[0:64, 6 + g] = wEk[:, 2 * g].reshape(64, 88)
            if 2 * g + 1 <= 10:
                wE[64:128, g] = wEq[:, 2 * g + 1].reshape(64, 88)
                wE[64:128, 6 + g] = wEk[:, 2 * g + 1].reshape(64, 88)
        blob = np.zeros((128, C_TOT), np.float16)
        blob[:, C_WEQ:C_F2A] = wE[:, 0:6].reshape(128, 528)
        blob[:, C_F2A:C_WEK] = f2[:, 0:34, :].reshape(128, 1224)
        blob[:, C_WEK:C_F2B] = wE[:, 6:12].reshape(128, 528)
        blob[:, C_F2B:C_WS] = f2[:, 34:67, :].reshape(128, 1188)
        blob[0:88, C_WS:C_TOT] = eye88
        w2 = np.zeros((128, 16), np.float32)
        w2[0:64, 0:8] = Wv[sl].T
        w2[64:128, 0:8] = Wv[sl].T
        w2[0:8, 8] = bq[sl]
        w2[0:8, 9] = bk[sl]
        in_maps.append({"blob": blob, "w2": w2})

    nc = _get_nc()
    res = run_bass_kernel_spmd(nc, in_maps, core_ids=list(range(N_CORES)))
    LAST_RESULTS = res
    out = np.concatenate([r["out"] for r in res.results], axis=0)  # [64, 4096]
    return out.reshape(1, 64, 64, 64)


# revision 10
# speedup vs baseline: 1.3981x; 1.0684x over previous
"""Trainium2 Bass kernel for nn_Attention_32049045963483 (sparse_attention).

Math collapse (validated vs reference at ~4e-4 l2 rel err):
  - qkv 1x1 conv folds into the 11x11/stride-8 down-convs HOST-SIDE:
      conv(W1 @ f, wq) == conv(f, w_eff)   (weight preprocessing)
  - 64x nearest upsample + softmax == softmax of the low-res score map;
    every output row depends only on x = i//64.
  - v enters only through 64-wide block sums: vbar = Wv @ fbar,
      fbar[d,J] = sum_y f[d,J,y]  (v never materializes)
  - out[c,x,y] = (sum_J e[J,x] vbar[J,c]) / (64 sum_J e[J,x]), broadcast on y.
  - exp via the tanh table (exact identity): e^x = 2/(1 - tanh(x/2)) - 1.
    tanh shares an ACT table with gelu's load set, so no mid-kernel 1.3us
    table reload (Exp would force one: the ACT table slots reload on every
    func-set switch).  Safe: scale*dots in [-0.5, 2.7].

Sharding: head-parallel over 8 cores; core i takes head i.  Each core reads
full f (the down-convs mix all 64 channels).

Stage 1 packs TWO kx taps per matmul into the full 128-partition contraction.
f2E is a phase-major permutation of padded f: rows 0..63 hold EVEN columns
(c=2m) laid out as [r, m%4, m//4], rows 64..127 the ODD columns (the +1 tap).
This gives a contiguous 8-element inner dim (fp16 needs contiguity for
1 col/cycle; strided-8 fp16 measured 2x slower) and stores each element once.
fbar drops out as the full 36-slot row sums of BOTH partition halves (pads
are zero); Wv is stacked twice in the vbar contraction so no half-add needed.

All inputs ride in ONE dram blob with just 2 DMA pieces per HWDGE ring
(partition-split).  More pieces choke shared descriptor engine 79 (measured:
with 10 input triggers its per-piece share completed 2.5-3us after the other
15 engines, delaying every completion semaphore).
"""

import numpy as np

N_CORES = 8
SCALE = 8.0 ** -0.5  # dim_head ** -0.5

# blob column map (fp16 elements)
C_WEQ = 0
C_F2A = 528
C_WEK = 1752
C_F2B = 2280
C_WS = 3468
C_TOT = 3556

_CACHE = {}

LAST_RESULTS = None  # BassKernelResults of the most recent run (for test harness)


def _dep(after, before, sync=False):
    from concourse.tile import add_dep_helper

    a = getattr(after, "ins", after)
    b = getattr(before, "ins", before)
    add_dep_helper(a, b, sync=sync, reason="pin order")


def _build_nc():
    from contextlib import ExitStack

    import concourse.bacc as bacc
    import concourse.bass as bass
    import concourse.mybir as mybir
    import concourse.tile as tile

    f32 = mybir.dt.float32
    f16 = mybir.dt.float16
    X = mybir.AxisListType.X
    AF = mybir.ActivationFunctionType
    ALU = mybir.AluOpType

    nc = bacc.Bacc("TRN2", target_bir_lowering=False)

    blob_d = nc.dram_tensor("blob", [128, C_TOT], f16, kind="ExternalInput")
    w2_d = nc.dram_tensor("w2", [128, 16], f32, kind="ExternalInput")
    out_d = nc.dram_tensor("out", [8, 4096], f32, kind="ExternalOutput")

    with tile.TileContext(nc) as tc:
        with ExitStack() as ctx:
            sb = ctx.enter_context(tc.tile_pool(name="sb", bufs=1))
            ps = ctx.enter_context(tc.tile_pool(name="ps", bufs=1, space="PSUM"))

            blob_t = sb.tile([128, C_TOT], f16)
            warm_t = sb.tile([128, 384], f16)
            w2_t = sb.tile([128, 16], f32)
            s_t = sb.tile([88, 67 * 16], f16)
            qk_t = sb.tile([8, 128], f16)
            th_t = sb.tile([64, 64], f32)
            u_t = sb.tile([64, 64], f32)
            e_t = sb.tile([64, 64], f16)
            fb_t = sb.tile([128, 64], f32)
            vaug_t = sb.tile([64, 9], f16)
            rs_t = sb.tile([64, 1], f32)
            T_t = sb.tile([64, 8 * 64], f32)
            scr_t = sb.tile([1, 1], f32)
            scr2_t = sb.tile([1, 1], f32)

            # --- input DMAs: 2 pieces per ring (partition-split), then w2
            d_p1s = nc.sync.dma_start(
                out=blob_t[0:64, 0:C_WEK], in_=blob_d[0:64, 0:C_WEK]
            )
            d_p1c = nc.scalar.dma_start(
                out=blob_t[64:128, 0:C_WEK], in_=blob_d[64:128, 0:C_WEK]
            )
            d_p2s = nc.sync.dma_start(
                out=blob_t[0:64, C_WEK:C_TOT], in_=blob_d[0:64, C_WEK:C_TOT]
            )
            d_p2c = nc.scalar.dma_start(
                out=blob_t[64:128, C_WEK:C_TOT], in_=blob_d[64:128, C_WEK:C_TOT]
            )
            d_w2 = nc.sync.dma_start(out=w2_t, in_=w2_d[:])

            # constants + ACT warm-up: dummy Gelu forces its table load early;
            # the load pass hoists tanh's set load next to it (measured).
            nc.vector.memset(scr_t, 0.0)
            nc.vector.memset(vaug_t[:, 8:9], 64.0)
            dg = nc.scalar.activation(out=scr2_t, in_=scr_t, func=AF.Gelu)
            dt = nc.scalar.activation(out=scr2_t, in_=scr_t, func=AF.Tanh)
            _dep(dg, d_p2c)
            _dep(dt, dg)

            # PE warm-up: ~3.8us of dummy matmuls during the DMA wait ramp
            # the Tensor engine clock (measured: cold mm pitch 0.83 ns/col,
            # warmed 0.43).  One stationary, 18 moving passes, junk PSUM.
            nc.vector.memset(warm_t, 0.0)
            ps_w = ps.tile([128, 256], f32, tag="H")
            for w in range(18):
                nc.tensor.matmul(
                    ps_w, warm_t[:, 0:128], warm_t[:, 128:384],
                    start=(w == 0), stop=(w == 17),
                )

            f23A = blob_t[:, C_F2A:C_WEK].rearrange("p (r s) -> p r s", s=36)
            f23B = blob_t[:, C_F2B:C_WS].rearrange("p (r s) -> p r s", s=36)
            ws_v = blob_t[0:88, C_WS:C_TOT]
            s3 = s_t.rearrange("p (r c16) -> p r c16", c16=16)
            wv2_v = w2_t[:, 0:8]
            bq_v = w2_t[0:8, 8:9]
            bk_v = w2_t[0:8, 9:10]

            # --- stage 1: 2 r-chunks x 2 convs x 6 kx-pairs, fp16, 128-deep
            ps_Aq = ps.tile([88, 34 * 8], f32, tag="A")
            ps_Ak = ps.tile([88, 34 * 8], f32, tag="B")
            ps_Bq = ps.tile([88, 33 * 8], f32, tag="C")
            ps_Bk = ps.tile([88, 33 * 8], f32, tag="D")

            def s1(f23c, pst, wbase):
                for g in range(6):
                    base = (g % 4) * 9 + (g // 4)
                    nc.tensor.matmul(
                        pst,
                        blob_t[:, wbase + g * 88 : wbase + g * 88 + 88],
                        f23c[:, :, base : base + 8],
                        start=(g == 0),
                        stop=(g == 5),
                    )

            def s1cast(eng, pst, rsl, cv):
                pin = pst.rearrange("p (r ox) -> p r ox", ox=8)
                out = s3[:, rsl, 8 * cv : 8 * cv + 8]
                if eng == "v":
                    nc.vector.tensor_copy(out=out, in_=pin)
                else:
                    nc.scalar.copy(out=out, in_=pin)

            slA, slB = slice(0, 34), slice(34, 67)

            # DVE order: redA, castqA, redB, castqB, vaug, tanh-chain, out ops
            # ACT order: dummies, castkA, castkB, gelu q/k, tanh
            s1(f23A, ps_Aq, C_WEQ)
            nc.vector.reduce_sum(out=fb_t[:, 0:32], in_=f23A[:, 2:34, :], axis=X)
            s1cast("v", ps_Aq, slA, 0)
            s1(f23A, ps_Ak, C_WEK)
            s1cast("a", ps_Ak, slA, 1)
            s1(f23B, ps_Bq, C_WEQ)
            nc.vector.reduce_sum(out=fb_t[:, 32:64], in_=f23B[:, 0:32, :], axis=X)
            s1cast("v", ps_Bq, slB, 0)
            s1(f23B, ps_Bk, C_WEK)
            s1cast("a", ps_Bk, slB, 1)

            # --- stage 2: 11 ky-selection matmuls, q+k fused (128 cols each)
            psc = ps.tile([8, 128], f32, tag="E")
            for ky in range(11):
                a = s_t[:, ky * 16 : 1072]
                mv = bass.AP(
                    tensor=a.tensor,
                    offset=a.offset,
                    ap=[list(a.ap[0]), [128, 8], [1, 16]],
                )
                nc.tensor.matmul(
                    psc,
                    ws_v[:, ky * 8 : ky * 8 + 8],
                    mv,
                    start=(ky == 0),
                    stop=(ky == 10),
                )

            # --- vbar: psv[J, c] = sum over 128 rows of fb * [Wv.T; Wv.T]
            psv = ps.tile([64, 8], f32, tag="G")
            nc.tensor.matmul(psv, fb_t, wv2_v, start=True, stop=True)
            nc.vector.tensor_copy(out=vaug_t[:, 0:8], in_=psv)

            # --- gelu (exact, table) on q and k halves of psc -> fp16 qk
            qk3 = qk_t.rearrange("p (b x) -> p b x", x=8)
            aq = psc[:, 0:128]
            gin_q = bass.AP(
                tensor=aq.tensor, offset=aq.offset,
                ap=[list(aq.ap[0]), [16, 8], [1, 8]],
            )
            ak = psc[:, 8:128]
            gin_k = bass.AP(
                tensor=ak.tensor, offset=ak.offset,
                ap=[list(ak.ap[0]), [16, 8], [1, 8]],
            )
            nc.scalar.activation(
                out=qk3[:, 0:8, :], in_=gin_q, func=AF.Gelu, bias=bq_v, scale=1.0
            )
            nc.scalar.activation(
                out=qk3[:, 8:16, :], in_=gin_k, func=AF.Gelu, bias=bk_v, scale=1.0
            )

            # --- dots^T[J, I] (fp16, single-pass) then e via tanh identity
            psd = ps.tile([64, 64], f32, tag="F")
            nc.tensor.matmul(
                psd, qk_t[:, 64:128], qk_t[:, 0:64], start=True, stop=True
            )
            nc.scalar.activation(out=th_t, in_=psd, func=AF.Tanh, scale=SCALE * 0.5)
            nc.vector.tensor_scalar(
                out=u_t, in0=th_t, scalar1=-1.0, scalar2=1.0,
                op0=ALU.mult, op1=ALU.add,
            )
            nc.vector.reciprocal(out=th_t, in_=u_t)
            nc.vector.tensor_scalar(
                out=e_t, in0=th_t, scalar1=2.0, scalar2=1.0,
                op0=ALU.mult, op1=ALU.subtract,
            )

            # --- out_u[I, 0:8] = sum_J e[J,I] vaug[J,:]; col 8 = 64*sum_J e
            pso = ps.tile([64, 9], f32, tag="H")
            nc.tensor.matmul(pso, e_t, vaug_t, start=True, stop=True)
            nc.vector.reciprocal(out=rs_t, in_=pso[:, 8:9])

            # --- normalize + broadcast along y, split by channel halves so
            # each half's DMA (on its own ring) launches as soon as its DVE
            # op lands; store out[c, x, y] <- T[x, c, y]
            T3 = T_t.rearrange("p (c y) -> p c y", y=64)
            out_ap = out_d[:].rearrange("c (x y) -> c x y", y=64).transpose([1, 0, 2])
            for h, dma_eng in ((0, nc.sync), (1, nc.scalar)):
                ao = pso[:, 4 * h : 4 * h + 4]
                o_b = bass.AP(
                    tensor=ao.tensor, offset=ao.offset,
                    ap=[list(ao.ap[0]), [1, 4], [0, 64]],
                )
                nc.vector.tensor_scalar_mul(T3[:, 4 * h : 4 * h + 4, :], o_b, rs_t)
                dma_eng.dma_start(
                    out=out_ap[:, 4 * h : 4 * h + 4, :],
                    in_=T3[:, 4 * h : 4 * h + 4, :],
                )

    nc.finalize()
    return nc


def _get_nc():
    if "nc" not in _CACHE:
        _CACHE["nc"] = _build_nc()
    return _CACHE["nc"]


def kernel(**inputs):
    global LAST_RESULTS
    from concourse.bass_utils import run_bass_kernel_spmd

    f = np.ascontiguousarray(inputs["f"], np.float32)
    w_qkv = np.ascontiguousarray(inputs["w_qkv"], np.float32)[:, :, 0, 0]  # [192,64]
    wq = np.ascontiguousarray(inputs["wq"], np.float32)
    wk = np.ascontiguousarray(inputs["wk"], np.float32)
    bq = np.ascontiguousarray(inputs["bq"], np.float32)
    bk = np.ascontiguousarray(inputs["bk"], np.float32)

    W1q, W1k, Wv = w_qkv[0:64], w_qkv[64:128], w_qkv[128:192]

    # f2E phase-major permutation: slot s = (m%4)*9 + m//4 holds column 2m
    # (rows 0..63) / column 2m+1 (rows 64..127) of the padded f.
    fpad = np.zeros((64, 68, 68), np.float32)
    fpad[:, 2:66, 2:66] = f[0]
    f2 = np.zeros((128, 67, 36), np.float32)
    for m in range(34):
        s = (m % 4) * 9 + m // 4
        f2[0:64, :, s] = fpad[:, 0:67, 2 * m]
        if 2 * m + 1 <= 67:
            f2[64:128, :, s] = fpad[:, 0:67, 2 * m + 1]
    f2 = f2.astype(np.float16)

    eye88 = np.eye(88, dtype=np.float16)

    in_maps = []
    for i in range(N_CORES):
        sl = slice(8 * i, 8 * i + 8)
        # w_eff[d, kx, ky, oc] = sum_ic wq[oc,ic,ky,kx] W1[ic,d]
        wEq = np.einsum("oiyx,id->dxyo", wq[sl], W1q)
        wEk = np.einsum("oiyx,id->dxyo", wk[sl], W1k)
        wE = np.zeros((128, 12, 88), np.float16)
        for g in range(6):
            wE[0:64, g] = wEq[:, 2 * g].reshape(64, 88)
            wE[0:64, 6 + g] = wEk[:, 2 * g].reshape(64, 88)
            if 2 * g + 1 <= 10:
                wE[64:128, g] = wEq[:, 2 * g + 1].reshape(64, 88)
                wE[64:128, 6 + g] = wEk[:, 2 * g + 1].reshape(64, 88)
        blob = np.zeros((128, C_TOT), np.float16)
        blob[:, C_WEQ:C_F2A] = wE[:, 0:6].reshape(128, 528)
        blob[:, C_F2A:C_WEK] = f2[:, 0:34, :].reshape(128, 1224)
        blob[:, C_WEK:C_F2B] = wE[:, 6:12].reshape(128, 528)
        blob[:, C_F2B:C_WS] = f2[:, 34:67, :].reshape(128, 1188)
        blob[0:88, C_WS:C_TOT] = eye88
        w2 = np.zeros((128, 16), np.float32)
        w2[0:64, 0:8] = Wv[sl].T
        w2[64:128, 0:8] = Wv[sl].T
        w2[0:8, 8] = bq[sl]
        w2[0:8, 9] = bk[sl]
        in_maps.append({"blob": blob, "w2": w2})

    nc = _get_nc()
    res = run_bass_kernel_spmd(nc, in_maps, core_ids=list(range(N_CORES)))
    LAST_RESULTS = res
    out = np.concatenate([r["out"] for r in res.results], axis=0)  # [64, 4096]
    return out.reshape(1, 64, 64, 64)
